# revision 4
# baseline (speedup 1.0000x reference)
"""
AdaptiveMessagePassingLayer Trainium2 kernel.

Math: out = inputs @ W_eff,  W_eff = sum_r relation_weights[r] * relation_scales[r]
Shapes: inputs [500000, 128] f32, relation_weights [8, 128, 128] f32,
        relation_scales [8, 1] f32  ->  out [500000, 128] f32.

Strategy (data-parallel over 8 NeuronCores, no comm):
  - Pad the node axis to 8 * SHARD rows, one shard per core.
  - Per core: compute W_eff once on-device (DVE scale+add), then stream the
    shard in CHUNK-row DMA chunks. Per 128-node tile: PE transpose (X tile is
    the stationary operand, identity streams) -> X^T in PSUM -> DVE copy to
    SBUF -> PE matmul (lhsT = X^T, rhs = W_eff) -> OUT tile natural layout in
    PSUM -> ACT copy to SBUF -> DMA out. Grouped 4 tiles per PSUM bank so the
    PSUM->SBUF copies are [128, 512].
  - Memory-bound problem: DMA in/out (~64 MB per core) is the roofline.
"""

import numpy as np

N_CORES = 8
D = 128
R = 8
TILE = 128
SUPER = 512               # 4 tiles share one PSUM bank / one copy
CHUNK = 2048              # rows per DMA chunk (1 MiB)
SHARD = 62976             # 123 supertiles; 8*62976 = 503808 >= 500000

_CACHE = {}


def _build_nc(shard_rows, chunk_rows):
    import concourse.mybir as mybir
    import concourse.tile as tile
    from concourse import bacc
    from concourse.masks import make_identity

    assert shard_rows % SUPER == 0

    nc = bacc.Bacc()
    x_ext = nc.declare_dram_parameter("x", [shard_rows, D], mybir.dt.float32, isOutput=False)
    rw_ext = nc.declare_dram_parameter("rw", [R, D, D], mybir.dt.float32, isOutput=False)
    rs_ext = nc.declare_dram_parameter("rs", [R, 1], mybir.dt.float32, isOutput=False)
    out_ext = nc.declare_dram_parameter("out", [shard_rows, D], mybir.dt.float32, isOutput=True)

    with tile.TileContext(nc) as tc:
        with (
            tc.tile_pool(name="const", bufs=1) as const_pool,
            tc.tile_pool(name="xin", bufs=2) as x_pool,
            tc.tile_pool(name="xt", bufs=3) as xt_pool,
            tc.tile_pool(name="oout", bufs=2) as o_pool,
            tc.tile_pool(name="tpsum", bufs=3, space="PSUM") as tr_pool,
            tc.tile_pool(name="mpsum", bufs=3, space="PSUM") as mm_pool,
        ):
            ident = const_pool.tile([D, D], mybir.dt.float32)
            make_identity(nc, ident[:])

            # W_eff = sum_r rw[r] * rs[r]
            w_all = const_pool.tile([D, R, D], mybir.dt.float32)
            nc.sync.dma_start(w_all[:], rw_ext[:, :, :].rearrange("r k m -> k r m"))
            s_row = const_pool.tile([1, R], mybir.dt.float32)
            nc.sync.dma_start(s_row[:], rs_ext[:, :].rearrange("r o -> o r"))
            s_bc = const_pool.tile([D, R], mybir.dt.float32)
            nc.gpsimd.partition_broadcast(s_bc[:], s_row[0:1, :])
            w_eff = const_pool.tile([D, D], mybir.dt.float32)
            w_tmp = const_pool.tile([D, D], mybir.dt.float32)
            nc.vector.tensor_scalar_mul(w_eff[:], w_all[:, 0, :], s_bc[:, 0:1])
            for r in range(1, R):
                nc.vector.tensor_scalar_mul(w_tmp[:], w_all[:, r, :], s_bc[:, r : r + 1])
                nc.vector.tensor_add(w_eff[:], w_eff[:], w_tmp[:])

            for c0 in range(0, shard_rows, chunk_rows):
                rows = min(chunk_rows, shard_rows - c0)
                assert rows % SUPER == 0
                ntiles = rows // TILE
                nsuper = rows // SUPER

                x_t = x_pool.tile([TILE, ntiles, D], mybir.dt.float32, tag="x")
                nc.sync.dma_start(
                    x_t[:], x_ext[c0 : c0 + rows, :].rearrange("(t p) d -> p t d", p=TILE)
                )
                o_t = o_pool.tile([TILE, ntiles, D], mybir.dt.float32, tag="o")

                for s in range(nsuper):
                    tr_ps = tr_pool.tile([TILE, 4, TILE], mybir.dt.float32, tag="trp")
                    for u in range(4):
                        nc.tensor.transpose(tr_ps[:, u, :], x_t[:, s * 4 + u, :], ident[:])
                    xt_t = xt_pool.tile([TILE, 4, TILE], mybir.dt.float32, tag="xt")
                    nc.vector.tensor_copy(xt_t[:], tr_ps[:])
                    mm_ps = mm_pool.tile([TILE, 4, TILE], mybir.dt.float32, tag="mmp")
                    for u in range(4):
                        nc.tensor.matmul(mm_ps[:, u, :], xt_t[:, u, :], w_eff[:])
                    nc.scalar.copy(o_t[:, s * 4 : s * 4 + 4, :], mm_ps[:])

                nc.scalar.dma_start(
                    out_ext[c0 : c0 + rows, :].rearrange("(t p) d -> p t d", p=TILE), o_t[:]
                )

    nc.finalize()
    return nc


def _get_nc(shard_rows=None, chunk_rows=None):
    shard_rows = SHARD if shard_rows is None else shard_rows
    chunk_rows = CHUNK if chunk_rows is None else chunk_rows
    key = (shard_rows, chunk_rows)
    if key not in _CACHE:
        _CACHE[key] = _build_nc(shard_rows, chunk_rows)
    return _CACHE[key]


def _run(inputs, relation_weights, relation_scales, trace=False):
    from concourse.bass_utils import run_bass_kernel_spmd

    x = np.ascontiguousarray(np.asarray(inputs, dtype=np.float32))
    rw = np.ascontiguousarray(np.asarray(relation_weights, dtype=np.float32))
    rs = np.ascontiguousarray(np.asarray(relation_scales, dtype=np.float32))
    n_in = x.shape[0]

    total = SHARD * N_CORES
    assert total >= n_in
    xp = np.zeros((total, D), dtype=np.float32)
    xp[:n_in] = x
    shards = xp.reshape(N_CORES, SHARD, D)

    in_maps = [
        {"x": np.ascontiguousarray(shards[i]), "rw": rw, "rs": rs} for i in range(N_CORES)
    ]
    nc = _get_nc()
    res = run_bass_kernel_spmd(nc, in_maps, core_ids=list(range(N_CORES)), trace=trace)
    out = np.concatenate([res.results[i]["out"] for i in range(N_CORES)], axis=0)[:n_in]
    return out, res


def kernel(inputs, relation_weights, relation_scales):
    out, _ = _run(inputs, relation_weights, relation_scales, trace=False)
    return out


# revision 6
# speedup vs baseline: 1.0229x; 1.0229x over previous
"""
AdaptiveMessagePassingLayer Trainium2 kernel.

Math: out = inputs @ W_eff,  W_eff = sum_r relation_weights[r] * relation_scales[r]
Shapes: inputs [500000, 128] f32, relation_weights [8, 128, 128] f32,
        relation_scales [8, 1] f32  ->  out [500000, 128] f32.

Strategy (data-parallel over 8 NeuronCores, no comm):
  - Pad the node axis to 8 * SHARD rows, one shard per core.
  - Per core: compute W_eff once on-device (DVE scale+add), then stream the
    shard in CHUNK-row DMA chunks. Per 128-node tile: PE transpose (X tile is
    the stationary operand, identity streams) -> X^T in PSUM -> DVE copy to
    SBUF -> PE matmul (lhsT = X^T, rhs = W_eff) -> OUT tile natural layout in
    PSUM -> ACT copy to SBUF -> DMA out. Grouped 4 tiles per PSUM bank so the
    PSUM->SBUF copies are [128, 512].
  - Memory-bound problem: DMA in/out (~64 MB per core) is the roofline.
"""

import numpy as np

N_CORES = 8
D = 128
R = 8
TILE = 128
SUPER = 512               # 4 tiles share one PSUM bank / one copy
CHUNK = 2048              # rows per DMA chunk (1 MiB)
SHARD = 62976             # 123 supertiles; 8*62976 = 503808 >= 500000

_CACHE = {}


def _build_nc(shard_rows, chunk_rows):
    import concourse.mybir as mybir
    import concourse.tile as tile
    from concourse import bacc
    from concourse.masks import make_identity

    assert shard_rows % SUPER == 0

    nc = bacc.Bacc()
    x_ext = nc.declare_dram_parameter("x", [shard_rows, D], mybir.dt.float32, isOutput=False)
    rw_ext = nc.declare_dram_parameter("rw", [R, D, D], mybir.dt.float32, isOutput=False)
    rs_ext = nc.declare_dram_parameter("rs", [R, 1], mybir.dt.float32, isOutput=False)
    out_ext = nc.declare_dram_parameter("out", [shard_rows, D], mybir.dt.float32, isOutput=True)

    with tile.TileContext(nc) as tc:
        with (
            tc.tile_pool(name="const", bufs=1) as const_pool,
            tc.tile_pool(name="xin", bufs=2) as x_pool,
            tc.tile_pool(name="xt", bufs=3) as xt_pool,
            tc.tile_pool(name="oout", bufs=2) as o_pool,
            tc.tile_pool(name="tpsum", bufs=3, space="PSUM") as tr_pool,
            tc.tile_pool(name="mpsum", bufs=3, space="PSUM") as mm_pool,
        ):
            BF16 = mybir.dt.bfloat16
            ident = const_pool.tile([D, D], BF16)
            make_identity(nc, ident[:])

            # W_eff = sum_r rw[r] * rs[r]  (f32 accumulate, then cast to bf16)
            w_all = const_pool.tile([D, R, D], mybir.dt.float32)
            nc.sync.dma_start(w_all[:], rw_ext[:, :, :].rearrange("r k m -> k r m"))
            s_row = const_pool.tile([1, R], mybir.dt.float32)
            nc.sync.dma_start(s_row[:], rs_ext[:, :].rearrange("r o -> o r"))
            s_bc = const_pool.tile([D, R], mybir.dt.float32)
            nc.gpsimd.partition_broadcast(s_bc[:], s_row[0:1, :])
            w_eff = const_pool.tile([D, D], mybir.dt.float32)
            w_tmp = const_pool.tile([D, D], mybir.dt.float32)
            nc.vector.tensor_scalar_mul(w_eff[:], w_all[:, 0, :], s_bc[:, 0:1])
            for r in range(1, R):
                nc.vector.tensor_scalar_mul(w_tmp[:], w_all[:, r, :], s_bc[:, r : r + 1])
                nc.vector.tensor_add(w_eff[:], w_eff[:], w_tmp[:])
            w_bf = const_pool.tile([D, D], BF16)
            nc.vector.tensor_copy(w_bf[:], w_eff[:])

            for c0 in range(0, shard_rows, chunk_rows):
                rows = min(chunk_rows, shard_rows - c0)
                assert rows % SUPER == 0
                ntiles = rows // TILE
                nsuper = rows // SUPER

                # layout: partition p holds rows [c0 + p*ntiles, c0 + (p+1)*ntiles)
                # -> per-partition DRAM runs of ntiles*512B (8KB) for the DMA.
                x_t = x_pool.tile([TILE, ntiles, D], BF16, tag="x")
                nc.gpsimd.dma_start(
                    x_t[:], x_ext[c0 : c0 + rows, :].rearrange("(p j) d -> p j d", j=ntiles)
                )
                o_t = o_pool.tile([TILE, ntiles, D], mybir.dt.float32, tag="o")

                for s in range(nsuper):
                    tr_ps = tr_pool.tile([TILE, 4, TILE], BF16, tag="trp")
                    for u in range(4):
                        nc.tensor.transpose(tr_ps[:, u, :], x_t[:, s * 4 + u, :], ident[:])
                    xt_t = xt_pool.tile([TILE, 4, TILE], BF16, tag="xt")
                    nc.vector.tensor_copy(xt_t[:], tr_ps[:])
                    mm_ps = mm_pool.tile([TILE, 4, TILE], mybir.dt.float32, tag="mmp")
                    for u in range(4):
                        nc.tensor.matmul(mm_ps[:, u, :], xt_t[:, u, :], w_bf[:])
                    nc.scalar.copy(o_t[:, s * 4 : s * 4 + 4, :], mm_ps[:])

                nc.scalar.dma_start(
                    out_ext[c0 : c0 + rows, :].rearrange("(p j) d -> p j d", j=ntiles), o_t[:]
                )

    nc.finalize()
    return nc


def _get_nc(shard_rows=None, chunk_rows=None):
    shard_rows = SHARD if shard_rows is None else shard_rows
    chunk_rows = CHUNK if chunk_rows is None else chunk_rows
    key = (shard_rows, chunk_rows)
    if key not in _CACHE:
        _CACHE[key] = _build_nc(shard_rows, chunk_rows)
    return _CACHE[key]


def _run(inputs, relation_weights, relation_scales, trace=False):
    from concourse.bass_utils import run_bass_kernel_spmd

    x = np.ascontiguousarray(np.asarray(inputs, dtype=np.float32))
    rw = np.ascontiguousarray(np.asarray(relation_weights, dtype=np.float32))
    rs = np.ascontiguousarray(np.asarray(relation_scales, dtype=np.float32))
    n_in = x.shape[0]

    total = SHARD * N_CORES
    assert total >= n_in
    xp = np.zeros((total, D), dtype=np.float32)
    xp[:n_in] = x
    shards = xp.reshape(N_CORES, SHARD, D)

    in_maps = [
        {"x": np.ascontiguousarray(shards[i]), "rw": rw, "rs": rs} for i in range(N_CORES)
    ]
    nc = _get_nc()
    res = run_bass_kernel_spmd(nc, in_maps, core_ids=list(range(N_CORES)), trace=trace)
    out = np.concatenate([res.results[i]["out"] for i in range(N_CORES)], axis=0)[:n_in]
    return out, res


def kernel(inputs, relation_weights, relation_scales):
    out, _ = _run(inputs, relation_weights, relation_scales, trace=False)
    return out
